# revision 14
# baseline (speedup 1.0000x reference)
"""Trainium2 Bass kernel for nn_CAN_Layer_74775380623980.

Math: with sequence length L=1, softmax over the single key is exactly 1.0
and the reference's masks are overwritten with ones, so the whole cross
attention collapses to

    E   = (protein @ Wv_p + drug @ Wv_d) / 2          # [N, 2048]
    out = concat([E, E], axis=1)                      # [N, 4096]

Sharding: pure data parallel, batch N=16384 split 8 ways (2048 rows/core);
the two V projection weights are replicated.

Precision/speed split: per tensor, K-strips 0..11 (1536 of 2048) run as
fp16 matmuls; strips 12..15 run as fp8-e4m3 DoubleRow matmuls (2 K-strips
per instruction at 2x PE rate). Both paths accumulate into the same PSUM
bank at a common scale of 2^16 (x scaled by 16, weights by 0.5*4096 — all
powers of two, exact), undone by a scaled PSUM->SBUF copy. Measured
end-to-end rel_fro error vs the fp32 reference: 1.9e-2 (< 2e-2 gate).
"""

import numpy as np
import ml_dtypes

P = 128          # partitions / systolic tile
N_FULL = 16384
D = 2048         # contraction dim per tensor
HID = 2048       # output dim per projection
NCORES = 8
M_SH = N_FULL // NCORES   # 2048 rows per core
KT = D // P               # 16 k-strips per tensor
J16 = 12                  # fp16 k-strips per tensor
DJ8 = (KT - J16) // 2     # fp8 DoubleRow steps per tensor (2 strips each)
NBLK = 512                # matmul free dim (one PSUM bank of fp32)
NB = HID // NBLK          # 4 n-blocks
MT_FULL = M_SH // P       # 16 m-tiles
XSCALE = 16.0             # x pre-scale (power of two)
WSCALE = 2048.0           # 0.5 (reference's /2) * 4096 weight pre-scale
OSCALE = 1.0 / (XSCALE * 4096.0)  # PSUM de-scale = 2^-16


def _build_module(mt_tiles=MT_FULL, reps=1, xbufs=2, obufs=2, paired=True,
                  j16=J16, dj8=DJ8, swi=False, adj=False):
    """reps>1 wraps the whole body in a device-side For_i — used only for
    wall-clock benchmarking (amplifies device time above RPC noise).
    j16/dj8 override the fp16/fp8 K-split (probing). swi uses
    DoubleRowSwInterleave (host pre-interleaves the stationary layout)."""
    import concourse.bass as bass  # noqa: F401
    import concourse.mybir as mybir
    import concourse.tile as tile
    from concourse import bacc

    fp16 = mybir.dt.float16
    fp8 = mybir.dt.float8e4
    f32 = mybir.dt.float32
    DR = (mybir.MatmulPerfMode.DoubleRowSwInterleave if swi
          else mybir.MatmulPerfMode.DoubleRow)

    nc = bacc.Bacc("TRN2", target_bir_lowering=False, debug=False)

    xp16_h = nc.dram_tensor("xp16", [mt_tiles, P, max(j16, 1), P], fp16, kind="ExternalInput")
    xd16_h = nc.dram_tensor("xd16", [mt_tiles, P, max(j16, 1), P], fp16, kind="ExternalInput")
    x8_hshape = ([mt_tiles, P, max(dj8, 1), 2 * P] if swi
                 else [mt_tiles, P, max(dj8, 1), 2, P])
    xp8_h = nc.dram_tensor("xp8", x8_hshape, fp8, kind="ExternalInput")
    xd8_h = nc.dram_tensor("xd8", x8_hshape, fp8, kind="ExternalInput")
    wp16_h = nc.dram_tensor("wp16", [max(j16, 1), P, HID], fp16, kind="ExternalInput")
    wd16_h = nc.dram_tensor("wd16", [max(j16, 1), P, HID], fp16, kind="ExternalInput")
    w8_hshape = ([max(dj8, 1), P, HID, 2] if adj
                 else [max(dj8, 1), P, 2, HID])
    wp8_h = nc.dram_tensor("wp8", w8_hshape, fp8, kind="ExternalInput")
    wd8_h = nc.dram_tensor("wd8", w8_hshape, fp8, kind="ExternalInput")
    out_h = nc.dram_tensor("out", [mt_tiles * P, HID], f32, kind="ExternalOutput")
    J16_, DJ8_ = j16, dj8

    with tile.TileContext(nc) as tc:
        with (
            tc.tile_pool(name="wpool", bufs=1) as wpool,
            tc.tile_pool(name="xpool", bufs=(2 * xbufs if paired else xbufs)) as xpool,
            tc.tile_pool(name="x8pool", bufs=4) as x8pool,
            tc.tile_pool(name="stpool", bufs=1) as stpool,
            tc.tile_pool(name="opool", bufs=obufs) as opool,
            tc.tile_pool(name="psum", bufs=(1 if paired else 2), space="PSUM") as pp,
        ):
            x_tiles = {}

            x8_shape = [P, DJ8_, 2 * P] if swi else [P, DJ8_, 2, P]

            def load_x(mt):
                tp16 = xpool.tile([P, J16_, P], fp16, tag="xp16", name=f"xp16_{mt}")
                nc.sync.dma_start(tp16[:], xp16_h[mt])
                td16 = xpool.tile([P, J16_, P], fp16, tag="xd16", name=f"xd16_{mt}")
                nc.sync.dma_start(td16[:], xd16_h[mt])
                x_tiles[mt] = (tp16, td16)

            def load_x8(mt):
                tp8 = x8pool.tile(x8_shape, fp8, tag="xp8", name=f"xp8_{mt}")
                nc.sync.dma_start(tp8[:], xp8_h[mt])
                td8 = x8pool.tile(x8_shape, fp8, tag="xd8", name=f"xd8_{mt}")
                nc.sync.dma_start(td8[:], xd8_h[mt])
                x8_tiles[mt] = (tp8, td8)

            next8 = [0]

            def ensure_loads8(upto):
                while next8[0] <= min(upto, mt_tiles - 1):
                    load_x8(next8[0])
                    next8[0] += 1

            w16_sb = {}
            w8_sb = {}

            def load_w16():
                # DMA order matches PE consumption order (P0, D0, P1, D1, ...)
                w16_sb.clear()
                for j in range(J16_):
                    for t, h in (("p", wp16_h), ("d", wd16_h)):
                        tw = wpool.tile([P, HID], fp16, tag=f"w16{t}{j}",
                                        name=f"w16{t}_{j}")
                        nc.sync.dma_start(tw[:], h[j])
                        w16_sb[t, j] = tw

            def load_w8():
                w8_sb.clear()
                w8_tshape = [P, HID, 2] if adj else [P, 2, HID]
                for t, h in (("p", wp8_h), ("d", wd8_h)) if DJ8_ else ():
                    for dj in range(DJ8_):
                        tw = wpool.tile(w8_tshape, fp8, tag=f"w8{t}{dj}",
                                        name=f"w8{t}_{dj}")
                        nc.sync.dma_start(tw[:], h[dj])
                        w8_sb[t, dj] = tw

            next_load = [1]

            def ensure_loads(upto):
                while next_load[0] <= min(upto, mt_tiles - 1):
                    load_x(next_load[0])
                    next_load[0] += 1

            def fp8_pass(mt_lo, mt_hi):
                # all DoubleRow matmuls for every m-tile, partial sums staged
                # to SBUF as de-scaled fp16 (values ~ E/4, well within fp16)
                for mt0 in range(mt_lo, mt_hi, 2):
                    pair = [mt0, mt0 + 1]
                    ensure_loads8(mt0 + 3)
                    psums = {
                        (h, nb): pp.tile(
                            [P, NBLK], f32, tag=f"ps{h}_{nb}", name=f"ps8_{mt0}_{h}_{nb}"
                        )
                        for h in range(2)
                        for nb in range(NB)
                    }
                    x8t = {mt: x8_tiles.pop(mt) for mt in pair}
                    for dj in range(DJ8_):
                        for ti, t in enumerate(("p", "d")):
                            first = dj == 0 and ti == 0
                            last = dj == DJ8_ - 1 and ti == 1
                            for h, mt in enumerate(pair):
                                for nb in range(NB):
                                    nc.tensor.matmul(
                                        psums[h, nb][:],
                                        x8t[mt][ti][:, dj],
                                        w8_sb[t, dj][:, :, nb * NBLK:(nb + 1) * NBLK],
                                        start=first,
                                        stop=last,
                                        perf_mode=DR,
                                    )
                    for h, mt in enumerate(pair):
                        for nb in range(NB):
                            st = stpool.tile([P, NBLK], fp16, tag=f"st{mt % 8}_{nb}",
                                             name=f"st_{mt}_{nb}")
                            nc.vector.tensor_scalar_mul(st[:], psums[h, nb][:], OSCALE)
                            st_tiles[mt, nb] = st

            def fp16_pass(mt_lo, mt_hi):
                for mt0 in range(mt_lo, mt_hi, 2):
                    pair = [mt0, mt0 + 1]
                    if J16_:
                        ensure_loads(mt0 + 1 + 2 * (xbufs - 1))
                        xt = {mt: x_tiles.pop(mt) for mt in pair}
                    psums = {
                        (h, nb): pp.tile(
                            [P, NBLK], f32, tag=f"ps{h}_{nb}", name=f"ps_{mt0}_{h}_{nb}"
                        )
                        for h in range(2)
                        for nb in range(NB)
                    }
                    for j in range(J16_):
                        for ti, t in enumerate(("p", "d")):
                            for h, mt in enumerate(pair):
                                for nb in range(NB):
                                    nc.tensor.matmul(
                                        psums[h, nb][:],
                                        xt[mt][ti][:, j, :],
                                        w16_sb[t, j][:, nb * NBLK:(nb + 1) * NBLK],
                                        start=(j == 0 and ti == 0),
                                        stop=(j == J16_ - 1 and ti == 1),
                                    )
                    for h, mt in enumerate(pair):
                        out_t = opool.tile([P, HID], f32, tag="out", name=f"out_{mt}")
                        for nb in range(NB):
                            if J16_ == 0:
                                nc.vector.tensor_copy(
                                    out_t[:, nb * NBLK:(nb + 1) * NBLK],
                                    st_tiles[mt, nb][:],
                                )
                            elif DJ8_:
                                nc.vector.scalar_tensor_tensor(
                                    out_t[:, nb * NBLK:(nb + 1) * NBLK],
                                    psums[h, nb][:],
                                    OSCALE,
                                    st_tiles[mt, nb][:],
                                    op0=mybir.AluOpType.mult,
                                    op1=mybir.AluOpType.add,
                                )
                            else:
                                nc.vector.tensor_scalar_mul(
                                    out_t[:, nb * NBLK:(nb + 1) * NBLK],
                                    psums[h, nb][:],
                                    OSCALE,
                                )
                        nc.sync.dma_start(out_h[mt * P:(mt + 1) * P, :], out_t[:])

            def body():
                assert mt_tiles % 4 == 0
                st_tiles.clear()
                x_tiles.clear()
                x8_tiles.clear()
                next8[0] = 0
                next_load[0] = 1
                if DJ8_:
                    load_w8()
                    ensure_loads8(3)
                if J16_:
                    load_x(0)
                    load_w16()
                half = mt_tiles // 2
                if DJ8_:
                    fp8_pass(0, half)
                    fp16_pass(0, half)
                    fp8_pass(half, mt_tiles)
                    fp16_pass(half, mt_tiles)
                else:
                    fp16_pass(0, mt_tiles)

            st_tiles = {}
            x8_tiles = {}
            if reps == 1:
                body()
            else:
                with tc.For_i(0, reps, 1):
                    body()

    nc.compile()
    return nc


def _q8(a):
    return a.astype(ml_dtypes.float8_e4m3)


def _prep_inputs(protein, drug, Wv_p, Wv_d, mt_tiles=MT_FULL,
                 j16=J16, dj8=DJ8, swi=False, adj=False):
    """Host-side shard + transpose-tile + dtype split/cast."""
    kcut = j16 * P

    def prep_w(W):
        W = np.asarray(W, dtype=np.float32) * WSCALE
        if j16:
            w16 = np.ascontiguousarray(
                W[:kcut].reshape(j16, P, HID).astype(np.float16))
        else:
            w16 = np.zeros((1, P, HID), np.float16)
        if dj8:
            # w8[dj, p, i, n] = W[kcut + (2dj+i)*P + p, n]
            w8 = W[kcut:kcut + dj8 * 2 * P].reshape(dj8, 2, P, HID)
            w8 = w8.transpose(0, 2, 1, 3)          # [dj, p, i, n]
            if adj:
                w8 = w8.transpose(0, 1, 3, 2)      # [dj, p, n, i]
            w8 = np.ascontiguousarray(_q8(w8))
        else:
            shape = (1, P, HID, 2) if adj else (1, P, 2, HID)
            w8 = np.zeros(shape, ml_dtypes.float8_e4m3)
        return w16, w8

    wp16, wp8 = prep_w(Wv_p)
    wd16, wd8 = prep_w(Wv_d)

    def tile_x(x):
        x = x * XSCALE
        # [rows, D] -> [mt, p, j, m]: t[mt,p,j,m] = x[mt*P+m, j*P+p]
        t = x.reshape(mt_tiles, P, KT, P).transpose(0, 3, 2, 1)
        if j16:
            t16 = np.ascontiguousarray(t[:, :, :j16, :].astype(np.float16))
        else:
            t16 = np.zeros((mt_tiles, P, 1, P), np.float16)
        if dj8:
            t8 = t[:, :, j16:j16 + 2 * dj8, :].reshape(mt_tiles, P, dj8, 2, P)
            if swi:
                # stored[p, dj, 2m'+i] = plane_i[p, 127-m'] (interleaved,
                # columns reversed) per DoubleRowSwInterleave convention
                rev = t8[:, :, :, :, ::-1]                   # [mt,p,dj,i,m']
                t8 = rev.transpose(0, 1, 2, 4, 3).reshape(mt_tiles, P, dj8, 2 * P)
            t8 = np.ascontiguousarray(_q8(t8))
        else:
            shape = (mt_tiles, P, 1, 2 * P) if swi else (mt_tiles, P, 1, 2, P)
            t8 = np.zeros(shape, ml_dtypes.float8_e4m3)
        return t16, t8

    protein = np.asarray(protein, dtype=np.float32)
    drug = np.asarray(drug, dtype=np.float32)
    in_maps = []
    rows = mt_tiles * P
    for c in range(NCORES):
        sl = slice(c * M_SH, c * M_SH + rows)
        xp16, xp8 = tile_x(protein[sl])
        xd16, xd8 = tile_x(drug[sl])
        in_maps.append(
            {
                "xp16": xp16, "xd16": xd16, "xp8": xp8, "xd8": xd8,
                "wp16": wp16, "wd16": wd16, "wp8": wp8, "wd8": wd8,
            }
        )
    return in_maps




# ---------------------------------------------------------------------------
# Strassen-1 on the fp16 portion (level-1 split of the [2048 x 3072] fp16
# GEMM into 7 products of [1024 x 1536] @ [1536 x 1024]), fp8 DoubleRow
# strips handled in separate per-column-group passes with fp16 staging.
# B-side Strassen combos are precomputed on the host and shipped as `wst`.

def _build_strassen_module(mt_tiles=MT_FULL, reps=1, j16=J16, dj8=DJ8):
    import concourse.bass as bass  # noqa: F401
    import concourse.mybir as mybir
    import concourse.tile as tile
    from concourse import bacc

    fp16 = mybir.dt.float16
    fp8 = mybir.dt.float8e4
    f32 = mybir.dt.float32
    DR = mybir.MatmulPerfMode.DoubleRow
    ADD = mybir.AluOpType.add
    SUB = mybir.AluOpType.subtract
    MULT = mybir.AluOpType.mult

    assert mt_tiles == 16 and j16 >= 1 and dj8 >= 1
    MH = mt_tiles // 2            # 8 row-tiles per row-half

    nc = bacc.Bacc("TRN2", target_bir_lowering=False, debug=False)

    xp16_h = nc.dram_tensor("xp16", [mt_tiles, P, j16, P], fp16, kind="ExternalInput")
    xd16_h = nc.dram_tensor("xd16", [mt_tiles, P, j16, P], fp16, kind="ExternalInput")
    xp8_h = nc.dram_tensor("xp8", [mt_tiles, P, dj8, 2, P], fp8, kind="ExternalInput")
    xd8_h = nc.dram_tensor("xd8", [mt_tiles, P, dj8, 2, P], fp8, kind="ExternalInput")
    wst_h = nc.dram_tensor("wst", [7, j16, P, HID // 2], fp16, kind="ExternalInput")
    wp8_h = nc.dram_tensor("wp8", [dj8, P, 2, HID], fp8, kind="ExternalInput")
    wd8_h = nc.dram_tensor("wd8", [dj8, P, 2, HID], fp8, kind="ExternalInput")
    out_h = nc.dram_tensor("out", [mt_tiles * P, HID], f32, kind="ExternalOutput")

    with tile.TileContext(nc) as tc:
        with (
            tc.tile_pool(name="bst", bufs=1) as bstpool,
            tc.tile_pool(name="w8pool", bufs=1) as w8pool,
            tc.tile_pool(name="xpool", bufs=2) as xpool,
            tc.tile_pool(name="x8pool", bufs=8) as x8pool,
            tc.tile_pool(name="apool", bufs=1) as apool,
            tc.tile_pool(name="stpool", bufs=1) as stpool,
            tc.tile_pool(name="opool", bufs=2) as opool,
            tc.tile_pool(name="psum", bufs=1, space="PSUM") as pp,
        ):
            w8_sb = {}

            def load_w8():
                for t, h in (("p", wp8_h), ("d", wd8_h)):
                    for dj in range(dj8):
                        tw = w8pool.tile([P, 2, HID], fp8, tag=f"w8{t}{dj}",
                                         name=f"w8{t}_{dj}")
                        nc.sync.dma_start(tw[:], h[dj])
                        w8_sb[t, dj] = tw

            def load_bst(u):
                # 7 B-combo slices [128, j16, 512] for this 512-col group
                tiles = []
                for i in range(7):
                    tw = bstpool.tile([P, j16, NBLK], fp16, tag=f"bst{i}",
                                      name=f"bst{i}_{u}")
                    for j in range(j16):
                        nc.sync.dma_start(
                            tw[:, j, :], wst_h[i, j][:, u * NBLK:(u + 1) * NBLK])
                    tiles.append(tw)
                return tiles

            x_tiles = {}
            next_x = [0]

            def load_x16(rp, gen):
                tags = (("pt", xp16_h, rp), ("pb", xp16_h, MH + rp),
                        ("dt", xd16_h, rp), ("db", xd16_h, MH + rp))
                tt = {}
                for tag, h, mt in tags:
                    t = xpool.tile([P, j16, P], fp16, tag=tag, name=f"{tag}_{gen}_{rp}")
                    nc.sync.dma_start(t[:], h[mt])
                    tt[tag] = t
                x_tiles[rp] = tt

            def ensure_x16(upto, gen):
                while next_x[0] <= min(upto, MH - 1):
                    load_x16(next_x[0], gen)
                    next_x[0] += 1

            x8_tiles = {}
            next_x8 = [0]

            def load_x8(mt, gen):
                tp8 = x8pool.tile([P, dj8, 2, P], fp8, tag="xp8",
                                  name=f"xp8_{gen}_{mt}")
                nc.sync.dma_start(tp8[:], xp8_h[mt])
                td8 = x8pool.tile([P, dj8, 2, P], fp8, tag="xd8",
                                  name=f"xd8_{gen}_{mt}")
                nc.sync.dma_start(td8[:], xd8_h[mt])
                x8_tiles[mt] = (tp8, td8)

            def ensure_x8(upto, gen):
                while next_x8[0] <= min(upto, mt_tiles - 1):
                    load_x8(next_x8[0], gen)
                    next_x8[0] += 1

            st_tiles = {}

            def fp8_pass(u, gen):
                # partials for column groups u (left half) and 2+u (right),
                # all 16 row tiles; staged to SBUF as de-scaled fp16
                next_x8[0] = 0
                x8_tiles.clear()
                for q0 in range(0, mt_tiles, 4):
                    quad = list(range(q0, q0 + 4))
                    ensure_x8(q0 + 7, gen)
                    x8t = {mt: x8_tiles.pop(mt) for mt in quad}
                    psums = {}
                    for qi, mt in enumerate(quad):
                        for si, nbg in enumerate((u, 2 + u)):
                            psums[mt, nbg] = pp.tile(
                                [P, NBLK], f32, tag=f"ps{2 * qi + si + 1}",
                                name=f"ps8_{gen}_{mt}_{nbg}")
                    for dj in range(dj8):
                        for ti, t in enumerate(("p", "d")):
                            first = dj == 0 and ti == 0
                            last = dj == dj8 - 1 and ti == 1
                            for mt in quad:
                                for nbg in (u, 2 + u):
                                    nc.tensor.matmul(
                                        psums[mt, nbg][:],
                                        x8t[mt][ti][:, dj],
                                        w8_sb[t, dj][:, :, nbg * NBLK:(nbg + 1) * NBLK],
                                        start=first,
                                        stop=last,
                                        perf_mode=DR,
                                    )
                    for mt in quad:
                        for si, nbg in enumerate((u, 2 + u)):
                            st = stpool.tile([P, NBLK], fp16, tag=f"st{mt}_{si}",
                                             name=f"st_{gen}_{mt}_{nbg}")
                            nc.vector.tensor_scalar_mul(
                                st[:], psums[mt, nbg][:], OSCALE)
                            st_tiles[mt, si] = st

            def strassen_pass(u, bst, gen):
                next_x[0] = 0
                x_tiles.clear()
                for rp in range(MH):
                    ensure_x16(rp + 1, gen)
                    xt = x_tiles.pop(rp)
                    pt, pb, dt, db = xt["pt"], xt["pb"], xt["dt"], xt["db"]
                    combos = [("a1", pt, db, ADD), ("a2", pb, db, ADD),
                              ("a5", pt, dt, ADD), ("a6", pb, pt, SUB),
                              ("a7", dt, db, SUB)]
                    at = {}
                    for tag, i0, i1, op in combos:
                        t = apool.tile([P, j16, P], fp16, tag=tag,
                                       name=f"{tag}_{gen}_{rp}")
                        if op is SUB:
                            # a6 = pb - pt, a7 = dt - db
                            nc.vector.tensor_tensor(t[:], i0[:], i1[:], op)
                        else:
                            nc.vector.tensor_tensor(t[:], i0[:], i1[:], op)
                        at[tag] = t
                    stats = [at["a1"], at["a2"], pt, db, at["a5"], at["a6"], at["a7"]]
                    ps = []
                    for i in range(7):
                        p_t = pp.tile([P, NBLK], f32, tag=f"ps{i + 1}",
                                      name=f"psS_{gen}_{rp}_{i}")
                        for j in range(j16):
                            nc.tensor.matmul(
                                p_t[:], stats[i][:, j, :], bst[i][:, j, :],
                                start=(j == 0), stop=(j == j16 - 1))
                        ps.append(p_t)
                    m1, m2, m3, m4, m5, m6, m7 = ps
                    o = {}
                    for key, mt_row, col in (("o11", rp, u), ("o12", rp, 2 + u),
                                             ("o21", MH + rp, u),
                                             ("o22", MH + rp, 2 + u)):
                        o[key] = opool.tile([P, NBLK], f32, tag=key,
                                            name=f"{key}_{gen}_{rp}")
                    # ordered to free psum banks for the next iteration early
                    nc.vector.tensor_tensor(o["o11"][:], m1[:], m4[:], ADD)
                    nc.vector.tensor_tensor(o["o22"][:], m1[:], m2[:], SUB)
                    nc.vector.tensor_tensor(o["o21"][:], m2[:], m4[:], ADD)
                    nc.vector.tensor_tensor(o["o12"][:], m3[:], m5[:], ADD)
                    nc.vector.tensor_tensor(o["o22"][:], o["o22"][:], m3[:], ADD)
                    nc.vector.tensor_tensor(o["o11"][:], o["o11"][:], m5[:], SUB)
                    nc.vector.tensor_tensor(o["o22"][:], o["o22"][:], m6[:], ADD)
                    nc.vector.tensor_tensor(o["o11"][:], o["o11"][:], m7[:], ADD)
                    for key, mt_row, si in (("o11", rp, 0), ("o12", rp, 1),
                                            ("o21", MH + rp, 0),
                                            ("o22", MH + rp, 1)):
                        nbg = si * 2 + u
                        nc.vector.scalar_tensor_tensor(
                            o[key][:], o[key][:], OSCALE, st_tiles[mt_row, si][:],
                            op0=MULT, op1=ADD)
                        nc.sync.dma_start(
                            out_h[mt_row * P:(mt_row + 1) * P,
                                  nbg * NBLK:(nbg + 1) * NBLK],
                            o[key][:])

            def body(rep):
                st_tiles.clear()
                load_w8()
                for u in (0, 1):
                    gen = f"{rep}_{u}"
                    fp8_pass(u, gen)
                    bst = load_bst(u)
                    strassen_pass(u, bst, gen)

            if reps == 1:
                body(0)
            else:
                with tc.For_i(0, reps, 1):
                    body("r")

    nc.compile()
    return nc


def _prep_strassen(protein, drug, Wv_p, Wv_d, mt_tiles=MT_FULL, j16=J16, dj8=DJ8):
    kcut = j16 * P
    HH = HID // 2

    def prep_w8(W):
        W = np.asarray(W, dtype=np.float32) * WSCALE
        w8 = W[kcut:kcut + dj8 * 2 * P].reshape(dj8, 2, P, HID).transpose(0, 2, 1, 3)
        return np.ascontiguousarray(_q8(w8))

    WSp = np.asarray(Wv_p, dtype=np.float32) * WSCALE
    WSd = np.asarray(Wv_d, dtype=np.float32) * WSCALE
    B11, B12 = WSp[:kcut, :HH], WSp[:kcut, HH:]
    B21, B22 = WSd[:kcut, :HH], WSd[:kcut, HH:]
    combos = [B11 + B22, B11, B12 - B22, B21 - B11, B22, B11 + B12, B21 + B22]
    wst = np.stack([c.reshape(j16, P, HH) for c in combos]).astype(np.float16)
    wst = np.ascontiguousarray(wst)
    wp8 = prep_w8(Wv_p)
    wd8 = prep_w8(Wv_d)

    def tile_x(x):
        x = x * XSCALE
        t = x.reshape(mt_tiles, P, KT, P).transpose(0, 3, 2, 1)
        t16 = np.ascontiguousarray(t[:, :, :j16, :].astype(np.float16))
        t8 = t[:, :, j16:j16 + 2 * dj8, :].reshape(mt_tiles, P, dj8, 2, P)
        return t16, np.ascontiguousarray(_q8(t8))

    protein = np.asarray(protein, dtype=np.float32)
    drug = np.asarray(drug, dtype=np.float32)
    in_maps = []
    rows = mt_tiles * P
    for c in range(NCORES):
        sl = slice(c * M_SH, c * M_SH + rows)
        xp16, xp8 = tile_x(protein[sl])
        xd16, xd8 = tile_x(drug[sl])
        in_maps.append({"xp16": xp16, "xd16": xd16, "xp8": xp8, "xd8": xd8,
                        "wst": wst, "wp8": wp8, "wd8": wd8})
    return in_maps


_MODULE_CACHE = {}


def _run(protein, drug, Wv_p, Wv_d, trace=False, mt_tiles=MT_FULL):
    from concourse.bass_utils import run_bass_kernel_spmd

    nc = _MODULE_CACHE.get(mt_tiles)
    if nc is None:
        nc = _MODULE_CACHE[mt_tiles] = _build_module(mt_tiles)
    in_maps = _prep_inputs(protein, drug, Wv_p, Wv_d, mt_tiles)
    res = run_bass_kernel_spmd(nc, in_maps, list(range(NCORES)), trace=trace)
    E = np.concatenate(
        [np.asarray(r["out"], dtype=np.float32) for r in res.results], axis=0
    )
    return E, res


def kernel(
    protein,
    drug,
    mask_prot=None,
    mask_drug=None,
    Wq_p=None,
    Wk_p=None,
    Wv_p=None,
    Wq_d=None,
    Wk_d=None,
    Wv_d=None,
):
    E, _ = _run(protein, drug, Wv_p, Wv_d, trace=False)
    return np.concatenate([E, E], axis=1)


def kernel_profiled(**inputs):
    E, res = _run(
        inputs["protein"], inputs["drug"], inputs["Wv_p"], inputs["Wv_d"], trace=False
    )
    out = np.concatenate([E, E], axis=1)
    return out, res


# revision 16
# speedup vs baseline: 1.0202x; 1.0202x over previous
"""Trainium2 Bass kernel for nn_CAN_Layer_74775380623980.

Math: with sequence length L=1, softmax over the single key is exactly 1.0
and the reference's masks are overwritten with ones, so the whole cross
attention collapses to

    E   = (protein @ Wv_p + drug @ Wv_d) / 2          # [N, 2048]
    out = concat([E, E], axis=1)                      # [N, 4096]

Sharding: pure data parallel, batch N=16384 split 8 ways (2048 rows/core);
the two V projection weights are replicated.

Precision/speed split: per tensor, K-strips 0..11 (1536 of 2048) run as
fp16 matmuls; strips 12..15 run as fp8-e4m3 DoubleRow matmuls (2 K-strips
per instruction at 2x PE rate). Both paths accumulate into the same PSUM
bank at a common scale of 2^16 (x scaled by 16, weights by 0.5*4096 — all
powers of two, exact), undone by a scaled PSUM->SBUF copy. Measured
end-to-end rel_fro error vs the fp32 reference: 1.9e-2 (< 2e-2 gate).
"""

import numpy as np
import ml_dtypes

P = 128          # partitions / systolic tile
N_FULL = 16384
D = 2048         # contraction dim per tensor
HID = 2048       # output dim per projection
NCORES = 8
M_SH = N_FULL // NCORES   # 2048 rows per core
KT = D // P               # 16 k-strips per tensor
J16 = 12                  # fp16 k-strips per tensor
DJ8 = (KT - J16) // 2     # fp8 DoubleRow steps per tensor (2 strips each)
NBLK = 512                # matmul free dim (one PSUM bank of fp32)
NB = HID // NBLK          # 4 n-blocks
MT_FULL = M_SH // P       # 16 m-tiles
XSCALE = 16.0             # x pre-scale (power of two)
WSCALE = 2048.0           # 0.5 (reference's /2) * 4096 weight pre-scale
OSCALE = 1.0 / (XSCALE * 4096.0)  # PSUM de-scale = 2^-16


def _build_module(mt_tiles=MT_FULL, reps=1, xbufs=2, obufs=2, paired=True,
                  j16=J16, dj8=DJ8, swi=False, adj=False):
    """reps>1 wraps the whole body in a device-side For_i — used only for
    wall-clock benchmarking (amplifies device time above RPC noise).
    j16/dj8 override the fp16/fp8 K-split (probing). swi uses
    DoubleRowSwInterleave (host pre-interleaves the stationary layout)."""
    import concourse.bass as bass  # noqa: F401
    import concourse.mybir as mybir
    import concourse.tile as tile
    from concourse import bacc

    fp16 = mybir.dt.float16
    fp8 = mybir.dt.float8e4
    f32 = mybir.dt.float32
    DR = (mybir.MatmulPerfMode.DoubleRowSwInterleave if swi
          else mybir.MatmulPerfMode.DoubleRow)

    nc = bacc.Bacc("TRN2", target_bir_lowering=False, debug=False)

    xp16_h = nc.dram_tensor("xp16", [mt_tiles, P, max(j16, 1), P], fp16, kind="ExternalInput")
    xd16_h = nc.dram_tensor("xd16", [mt_tiles, P, max(j16, 1), P], fp16, kind="ExternalInput")
    x8_hshape = ([mt_tiles, P, max(dj8, 1), 2 * P] if swi
                 else [mt_tiles, P, max(dj8, 1), 2, P])
    xp8_h = nc.dram_tensor("xp8", x8_hshape, fp8, kind="ExternalInput")
    xd8_h = nc.dram_tensor("xd8", x8_hshape, fp8, kind="ExternalInput")
    wp16_h = nc.dram_tensor("wp16", [max(j16, 1), P, HID], fp16, kind="ExternalInput")
    wd16_h = nc.dram_tensor("wd16", [max(j16, 1), P, HID], fp16, kind="ExternalInput")
    w8_hshape = ([max(dj8, 1), P, HID, 2] if adj
                 else [max(dj8, 1), P, 2, HID])
    wp8_h = nc.dram_tensor("wp8", w8_hshape, fp8, kind="ExternalInput")
    wd8_h = nc.dram_tensor("wd8", w8_hshape, fp8, kind="ExternalInput")
    out_h = nc.dram_tensor("out", [mt_tiles * P, HID], f32, kind="ExternalOutput")
    J16_, DJ8_ = j16, dj8

    with tile.TileContext(nc) as tc:
        with (
            tc.tile_pool(name="wpool", bufs=1) as wpool,
            tc.tile_pool(name="xpool", bufs=(2 * xbufs if paired else xbufs)) as xpool,
            tc.tile_pool(name="x8pool", bufs=4) as x8pool,
            tc.tile_pool(name="stpool", bufs=1) as stpool,
            tc.tile_pool(name="opool", bufs=obufs) as opool,
            tc.tile_pool(name="psum", bufs=(1 if paired else 2), space="PSUM") as pp,
        ):
            x_tiles = {}

            x8_shape = [P, DJ8_, 2 * P] if swi else [P, DJ8_, 2, P]

            def load_x(mt):
                tp16 = xpool.tile([P, J16_, P], fp16, tag="xp16", name=f"xp16_{mt}")
                nc.sync.dma_start(tp16[:], xp16_h[mt])
                td16 = xpool.tile([P, J16_, P], fp16, tag="xd16", name=f"xd16_{mt}")
                nc.sync.dma_start(td16[:], xd16_h[mt])
                x_tiles[mt] = (tp16, td16)

            def load_x8(mt):
                tp8 = x8pool.tile(x8_shape, fp8, tag="xp8", name=f"xp8_{mt}")
                nc.sync.dma_start(tp8[:], xp8_h[mt])
                td8 = x8pool.tile(x8_shape, fp8, tag="xd8", name=f"xd8_{mt}")
                nc.sync.dma_start(td8[:], xd8_h[mt])
                x8_tiles[mt] = (tp8, td8)

            next8 = [0]

            def ensure_loads8(upto):
                while next8[0] <= min(upto, mt_tiles - 1):
                    load_x8(next8[0])
                    next8[0] += 1

            w16_sb = {}
            w8_sb = {}

            def load_w16():
                # DMA order matches PE consumption order (P0, D0, P1, D1, ...)
                w16_sb.clear()
                for j in range(J16_):
                    for t, h in (("p", wp16_h), ("d", wd16_h)):
                        tw = wpool.tile([P, HID], fp16, tag=f"w16{t}{j}",
                                        name=f"w16{t}_{j}")
                        nc.sync.dma_start(tw[:], h[j])
                        w16_sb[t, j] = tw

            def load_w8():
                w8_sb.clear()
                w8_tshape = [P, HID, 2] if adj else [P, 2, HID]
                for t, h in (("p", wp8_h), ("d", wd8_h)) if DJ8_ else ():
                    for dj in range(DJ8_):
                        tw = wpool.tile(w8_tshape, fp8, tag=f"w8{t}{dj}",
                                        name=f"w8{t}_{dj}")
                        nc.sync.dma_start(tw[:], h[dj])
                        w8_sb[t, dj] = tw

            next_load = [1]

            def ensure_loads(upto):
                while next_load[0] <= min(upto, mt_tiles - 1):
                    load_x(next_load[0])
                    next_load[0] += 1

            def fp8_pass(mt_lo, mt_hi):
                # all DoubleRow matmuls for every m-tile, partial sums staged
                # to SBUF as de-scaled fp16 (values ~ E/4, well within fp16)
                for mt0 in range(mt_lo, mt_hi, 2):
                    pair = [mt0, mt0 + 1]
                    ensure_loads8(mt0 + 3)
                    psums = {
                        (h, nb): pp.tile(
                            [P, NBLK], f32, tag=f"ps{h}_{nb}", name=f"ps8_{mt0}_{h}_{nb}"
                        )
                        for h in range(2)
                        for nb in range(NB)
                    }
                    x8t = {mt: x8_tiles.pop(mt) for mt in pair}
                    for dj in range(DJ8_):
                        for ti, t in enumerate(("p", "d")):
                            first = dj == 0 and ti == 0
                            last = dj == DJ8_ - 1 and ti == 1
                            for h, mt in enumerate(pair):
                                for nb in range(NB):
                                    nc.tensor.matmul(
                                        psums[h, nb][:],
                                        x8t[mt][ti][:, dj],
                                        w8_sb[t, dj][:, :, nb * NBLK:(nb + 1) * NBLK],
                                        start=first,
                                        stop=last,
                                        perf_mode=DR,
                                    )
                    for h, mt in enumerate(pair):
                        for nb in range(NB):
                            st = stpool.tile([P, NBLK], fp16, tag=f"st{mt % 8}_{nb}",
                                             name=f"st_{mt}_{nb}")
                            nc.vector.tensor_scalar_mul(st[:], psums[h, nb][:], OSCALE)
                            st_tiles[mt, nb] = st

            def fp16_pass(mt_lo, mt_hi):
                for mt0 in range(mt_lo, mt_hi, 2):
                    pair = [mt0, mt0 + 1]
                    if J16_:
                        ensure_loads(mt0 + 1 + 2 * (xbufs - 1))
                        xt = {mt: x_tiles.pop(mt) for mt in pair}
                    psums = {
                        (h, nb): pp.tile(
                            [P, NBLK], f32, tag=f"ps{h}_{nb}", name=f"ps_{mt0}_{h}_{nb}"
                        )
                        for h in range(2)
                        for nb in range(NB)
                    }
                    for j in range(J16_):
                        for ti, t in enumerate(("p", "d")):
                            for h, mt in enumerate(pair):
                                for nb in range(NB):
                                    nc.tensor.matmul(
                                        psums[h, nb][:],
                                        xt[mt][ti][:, j, :],
                                        w16_sb[t, j][:, nb * NBLK:(nb + 1) * NBLK],
                                        start=(j == 0 and ti == 0),
                                        stop=(j == J16_ - 1 and ti == 1),
                                    )
                    for h, mt in enumerate(pair):
                        out_t = opool.tile([P, HID], f32, tag="out", name=f"out_{mt}")
                        for nb in range(NB):
                            if J16_ == 0:
                                nc.vector.tensor_copy(
                                    out_t[:, nb * NBLK:(nb + 1) * NBLK],
                                    st_tiles[mt, nb][:],
                                )
                            elif DJ8_:
                                nc.vector.scalar_tensor_tensor(
                                    out_t[:, nb * NBLK:(nb + 1) * NBLK],
                                    psums[h, nb][:],
                                    OSCALE,
                                    st_tiles[mt, nb][:],
                                    op0=mybir.AluOpType.mult,
                                    op1=mybir.AluOpType.add,
                                )
                            else:
                                nc.vector.tensor_scalar_mul(
                                    out_t[:, nb * NBLK:(nb + 1) * NBLK],
                                    psums[h, nb][:],
                                    OSCALE,
                                )
                        nc.sync.dma_start(out_h[mt * P:(mt + 1) * P, :], out_t[:])

            def body():
                assert mt_tiles % 4 == 0
                st_tiles.clear()
                x_tiles.clear()
                x8_tiles.clear()
                next8[0] = 0
                next_load[0] = 1
                if DJ8_:
                    load_w8()
                    ensure_loads8(3)
                if J16_:
                    load_x(0)
                    load_w16()
                half = mt_tiles // 2
                if DJ8_:
                    fp8_pass(0, half)
                    fp16_pass(0, half)
                    fp8_pass(half, mt_tiles)
                    fp16_pass(half, mt_tiles)
                else:
                    fp16_pass(0, mt_tiles)

            st_tiles = {}
            x8_tiles = {}
            if reps == 1:
                body()
            else:
                with tc.For_i(0, reps, 1):
                    body()

    nc.compile()
    return nc


def _q8(a):
    return a.astype(ml_dtypes.float8_e4m3)


def _prep_inputs(protein, drug, Wv_p, Wv_d, mt_tiles=MT_FULL,
                 j16=J16, dj8=DJ8, swi=False, adj=False):
    """Host-side shard + transpose-tile + dtype split/cast."""
    kcut = j16 * P

    def prep_w(W):
        W = np.asarray(W, dtype=np.float32) * WSCALE
        if j16:
            w16 = np.ascontiguousarray(
                W[:kcut].reshape(j16, P, HID).astype(np.float16))
        else:
            w16 = np.zeros((1, P, HID), np.float16)
        if dj8:
            # w8[dj, p, i, n] = W[kcut + (2dj+i)*P + p, n]
            w8 = W[kcut:kcut + dj8 * 2 * P].reshape(dj8, 2, P, HID)
            w8 = w8.transpose(0, 2, 1, 3)          # [dj, p, i, n]
            if adj:
                w8 = w8.transpose(0, 1, 3, 2)      # [dj, p, n, i]
            w8 = np.ascontiguousarray(_q8(w8))
        else:
            shape = (1, P, HID, 2) if adj else (1, P, 2, HID)
            w8 = np.zeros(shape, ml_dtypes.float8_e4m3)
        return w16, w8

    wp16, wp8 = prep_w(Wv_p)
    wd16, wd8 = prep_w(Wv_d)

    def tile_x(x):
        x = x * XSCALE
        # [rows, D] -> [mt, p, j, m]: t[mt,p,j,m] = x[mt*P+m, j*P+p]
        t = x.reshape(mt_tiles, P, KT, P).transpose(0, 3, 2, 1)
        if j16:
            t16 = np.ascontiguousarray(t[:, :, :j16, :].astype(np.float16))
        else:
            t16 = np.zeros((mt_tiles, P, 1, P), np.float16)
        if dj8:
            t8 = t[:, :, j16:j16 + 2 * dj8, :].reshape(mt_tiles, P, dj8, 2, P)
            if swi:
                # stored[p, dj, 2m'+i] = plane_i[p, 127-m'] (interleaved,
                # columns reversed) per DoubleRowSwInterleave convention
                rev = t8[:, :, :, :, ::-1]                   # [mt,p,dj,i,m']
                t8 = rev.transpose(0, 1, 2, 4, 3).reshape(mt_tiles, P, dj8, 2 * P)
            t8 = np.ascontiguousarray(_q8(t8))
        else:
            shape = (mt_tiles, P, 1, 2 * P) if swi else (mt_tiles, P, 1, 2, P)
            t8 = np.zeros(shape, ml_dtypes.float8_e4m3)
        return t16, t8

    protein = np.asarray(protein, dtype=np.float32)
    drug = np.asarray(drug, dtype=np.float32)
    in_maps = []
    rows = mt_tiles * P
    for c in range(NCORES):
        sl = slice(c * M_SH, c * M_SH + rows)
        xp16, xp8 = tile_x(protein[sl])
        xd16, xd8 = tile_x(drug[sl])
        in_maps.append(
            {
                "xp16": xp16, "xd16": xd16, "xp8": xp8, "xd8": xd8,
                "wp16": wp16, "wd16": wd16, "wp8": wp8, "wd8": wd8,
            }
        )
    return in_maps




# ---------------------------------------------------------------------------
# Strassen-1 on the fp16 portion (level-1 split of the [2048 x 3072] fp16
# GEMM into 7 products of [1024 x 1536] @ [1536 x 1024]), fp8 DoubleRow
# strips handled in separate per-column-group passes with fp16 staging.
# B-side Strassen combos are precomputed on the host and shipped as `wst`.

def _build_strassen_module(mt_tiles=MT_FULL, reps=1, j16=J16, dj8=DJ8):
    import concourse.bass as bass  # noqa: F401
    import concourse.mybir as mybir
    import concourse.tile as tile
    from concourse import bacc

    fp16 = mybir.dt.float16
    fp8 = mybir.dt.float8e4
    f32 = mybir.dt.float32
    DR = mybir.MatmulPerfMode.DoubleRow
    ADD = mybir.AluOpType.add
    SUB = mybir.AluOpType.subtract
    MULT = mybir.AluOpType.mult

    assert mt_tiles == 16 and j16 >= 1 and dj8 >= 1
    MH = mt_tiles // 2            # 8 row-tiles per row-half

    nc = bacc.Bacc("TRN2", target_bir_lowering=False, debug=False)

    xp16_h = nc.dram_tensor("xp16", [mt_tiles, P, j16, P], fp16, kind="ExternalInput")
    xd16_h = nc.dram_tensor("xd16", [mt_tiles, P, j16, P], fp16, kind="ExternalInput")
    xp8_h = nc.dram_tensor("xp8", [mt_tiles, P, dj8, 2, P], fp8, kind="ExternalInput")
    xd8_h = nc.dram_tensor("xd8", [mt_tiles, P, dj8, 2, P], fp8, kind="ExternalInput")
    wst_h = nc.dram_tensor("wst", [7, j16, P, HID // 2], fp16, kind="ExternalInput")
    wp8_h = nc.dram_tensor("wp8", [dj8, P, 2, HID], fp8, kind="ExternalInput")
    wd8_h = nc.dram_tensor("wd8", [dj8, P, 2, HID], fp8, kind="ExternalInput")
    out_h = nc.dram_tensor("out", [mt_tiles * P, HID], f32, kind="ExternalOutput")

    with tile.TileContext(nc) as tc:
        with (
            tc.tile_pool(name="bst", bufs=1) as bstpool,
            tc.tile_pool(name="w8pool", bufs=1) as w8pool,
            tc.tile_pool(name="xpool", bufs=2) as xpool,
            tc.tile_pool(name="x8pool", bufs=8) as x8pool,
            tc.tile_pool(name="apool", bufs=1) as apool,
            tc.tile_pool(name="stpool", bufs=1) as stpool,
            tc.tile_pool(name="opool", bufs=2) as opool,
            tc.tile_pool(name="psum", bufs=1, space="PSUM") as pp,
        ):
            w8_sb = {}

            def load_w8():
                for t, h in (("p", wp8_h), ("d", wd8_h)):
                    for dj in range(dj8):
                        tw = w8pool.tile([P, 2, HID], fp8, tag=f"w8{t}{dj}",
                                         name=f"w8{t}_{dj}")
                        nc.sync.dma_start(tw[:], h[dj])
                        w8_sb[t, dj] = tw

            def load_bst(u):
                # 7 B-combo slices [128, j16, 512] for this 512-col group
                tiles = []
                for i in range(7):
                    tw = bstpool.tile([P, j16, NBLK], fp16, tag=f"bst{i}",
                                      name=f"bst{i}_{u}")
                    for j in range(j16):
                        nc.sync.dma_start(
                            tw[:, j, :], wst_h[i, j][:, u * NBLK:(u + 1) * NBLK])
                    tiles.append(tw)
                return tiles

            x_tiles = {}
            next_x = [0]

            def load_x16(rp, gen):
                tags = (("pt", xp16_h, rp), ("pb", xp16_h, MH + rp),
                        ("dt", xd16_h, rp), ("db", xd16_h, MH + rp))
                tt = {}
                for tag, h, mt in tags:
                    t = xpool.tile([P, j16, P], fp16, tag=tag, name=f"{tag}_{gen}_{rp}")
                    nc.sync.dma_start(t[:], h[mt])
                    tt[tag] = t
                x_tiles[rp] = tt

            def ensure_x16(upto, gen):
                while next_x[0] <= min(upto, MH - 1):
                    load_x16(next_x[0], gen)
                    next_x[0] += 1

            x8_tiles = {}
            next_x8 = [0]

            def load_x8(mt, gen):
                tp8 = x8pool.tile([P, dj8, 2, P], fp8, tag="xp8",
                                  name=f"xp8_{gen}_{mt}")
                nc.sync.dma_start(tp8[:], xp8_h[mt])
                td8 = x8pool.tile([P, dj8, 2, P], fp8, tag="xd8",
                                  name=f"xd8_{gen}_{mt}")
                nc.sync.dma_start(td8[:], xd8_h[mt])
                x8_tiles[mt] = (tp8, td8)

            def ensure_x8(upto, gen):
                while next_x8[0] <= min(upto, mt_tiles - 1):
                    load_x8(next_x8[0], gen)
                    next_x8[0] += 1

            st_tiles = {}

            def fp8_pass(u, gen):
                # partials for column groups u (left half) and 2+u (right),
                # all 16 row tiles; staged to SBUF as de-scaled fp16
                next_x8[0] = 0
                x8_tiles.clear()
                for q0 in range(0, mt_tiles, 4):
                    quad = list(range(q0, q0 + 4))
                    ensure_x8(q0 + 7, gen)
                    x8t = {mt: x8_tiles.pop(mt) for mt in quad}
                    psums = {}
                    for qi, mt in enumerate(quad):
                        for si, nbg in enumerate((u, 2 + u)):
                            psums[mt, nbg] = pp.tile(
                                [P, NBLK], f32, tag=f"ps{2 * qi + si + 1}",
                                name=f"ps8_{gen}_{mt}_{nbg}")
                    for dj in range(dj8):
                        for ti, t in enumerate(("p", "d")):
                            first = dj == 0 and ti == 0
                            last = dj == dj8 - 1 and ti == 1
                            for mt in quad:
                                for nbg in (u, 2 + u):
                                    nc.tensor.matmul(
                                        psums[mt, nbg][:],
                                        x8t[mt][ti][:, dj],
                                        w8_sb[t, dj][:, :, nbg * NBLK:(nbg + 1) * NBLK],
                                        start=first,
                                        stop=last,
                                        perf_mode=DR,
                                    )
                    for mt in quad:
                        for si, nbg in enumerate((u, 2 + u)):
                            st = stpool.tile([P, NBLK], fp16, tag=f"st{mt}_{si}",
                                             name=f"st_{gen}_{mt}_{nbg}")
                            nc.vector.tensor_scalar_mul(
                                st[:], psums[mt, nbg][:], OSCALE)
                            st_tiles[mt, si] = st

            def strassen_pass(u, bst, gen):
                next_x[0] = 0
                x_tiles.clear()
                for rp in range(MH):
                    ensure_x16(rp + 1, gen)
                    xt = x_tiles.pop(rp)
                    pt, pb, dt, db = xt["pt"], xt["pb"], xt["dt"], xt["db"]
                    combos = [("a1", pt, db, ADD), ("a2", pb, db, ADD),
                              ("a5", pt, dt, ADD), ("a6", pb, pt, SUB),
                              ("a7", dt, db, SUB)]
                    at = {}
                    for tag, i0, i1, op in combos:
                        t = apool.tile([P, j16, P], fp16, tag=tag,
                                       name=f"{tag}_{gen}_{rp}")
                        if op is SUB:
                            # a6 = pb - pt, a7 = dt - db
                            nc.vector.tensor_tensor(t[:], i0[:], i1[:], op)
                        else:
                            nc.vector.tensor_tensor(t[:], i0[:], i1[:], op)
                        at[tag] = t
                    stats = [at["a1"], at["a2"], pt, db, at["a5"], at["a6"], at["a7"]]
                    ps = []
                    for i in range(7):
                        p_t = pp.tile([P, NBLK], f32, tag=f"ps{i + 1}",
                                      name=f"psS_{gen}_{rp}_{i}")
                        for j in range(j16):
                            nc.tensor.matmul(
                                p_t[:], stats[i][:, j, :], bst[i][:, j, :],
                                start=(j == 0), stop=(j == j16 - 1))
                        ps.append(p_t)
                    m1, m2, m3, m4, m5, m6, m7 = ps
                    o = {}
                    for key, mt_row, col in (("o11", rp, u), ("o12", rp, 2 + u),
                                             ("o21", MH + rp, u),
                                             ("o22", MH + rp, 2 + u)):
                        o[key] = opool.tile([P, NBLK], f32, tag=key,
                                            name=f"{key}_{gen}_{rp}")
                    # DVE may read at most one PSUM operand per instruction:
                    # copy psum->sbuf once, then accumulate psum terms one at
                    # a time. Ordered to free psum banks early for the next
                    # iteration's products.
                    nc.vector.tensor_copy(o["o11"][:], m1[:])
                    nc.vector.tensor_scalar_mul(o["o22"][:], m2[:], -1.0)
                    nc.vector.tensor_tensor(o["o22"][:], o["o22"][:], m1[:], ADD)
                    nc.vector.tensor_copy(o["o21"][:], m2[:])
                    nc.vector.tensor_tensor(o["o11"][:], o["o11"][:], m4[:], ADD)
                    nc.vector.tensor_tensor(o["o21"][:], o["o21"][:], m4[:], ADD)
                    nc.vector.tensor_copy(o["o12"][:], m3[:])
                    nc.vector.tensor_tensor(o["o22"][:], o["o22"][:], m3[:], ADD)
                    nc.vector.tensor_tensor(o["o12"][:], o["o12"][:], m5[:], ADD)
                    nc.vector.tensor_tensor(o["o11"][:], o["o11"][:], m5[:], SUB)
                    nc.vector.tensor_tensor(o["o22"][:], o["o22"][:], m6[:], ADD)
                    nc.vector.tensor_tensor(o["o11"][:], o["o11"][:], m7[:], ADD)
                    for key, mt_row, si in (("o11", rp, 0), ("o12", rp, 1),
                                            ("o21", MH + rp, 0),
                                            ("o22", MH + rp, 1)):
                        nbg = si * 2 + u
                        nc.vector.scalar_tensor_tensor(
                            o[key][:], o[key][:], OSCALE, st_tiles[mt_row, si][:],
                            op0=MULT, op1=ADD)
                        nc.sync.dma_start(
                            out_h[mt_row * P:(mt_row + 1) * P,
                                  nbg * NBLK:(nbg + 1) * NBLK],
                            o[key][:])

            def body(rep):
                st_tiles.clear()
                load_w8()
                for u in (0, 1):
                    gen = f"{rep}_{u}"
                    fp8_pass(u, gen)
                    bst = load_bst(u)
                    strassen_pass(u, bst, gen)

            if reps == 1:
                body(0)
            else:
                with tc.For_i(0, reps, 1):
                    body("r")

    nc.compile()
    return nc


def _prep_strassen(protein, drug, Wv_p, Wv_d, mt_tiles=MT_FULL, j16=J16, dj8=DJ8):
    kcut = j16 * P
    HH = HID // 2

    def prep_w8(W):
        W = np.asarray(W, dtype=np.float32) * WSCALE
        w8 = W[kcut:kcut + dj8 * 2 * P].reshape(dj8, 2, P, HID).transpose(0, 2, 1, 3)
        return np.ascontiguousarray(_q8(w8))

    WSp = np.asarray(Wv_p, dtype=np.float32) * WSCALE
    WSd = np.asarray(Wv_d, dtype=np.float32) * WSCALE
    B11, B12 = WSp[:kcut, :HH], WSp[:kcut, HH:]
    B21, B22 = WSd[:kcut, :HH], WSd[:kcut, HH:]
    combos = [B11 + B22, B11, B12 - B22, B21 - B11, B22, B11 + B12, B21 + B22]
    wst = np.stack([c.reshape(j16, P, HH) for c in combos]).astype(np.float16)
    wst = np.ascontiguousarray(wst)
    wp8 = prep_w8(Wv_p)
    wd8 = prep_w8(Wv_d)

    def tile_x(x):
        x = x * XSCALE
        t = x.reshape(mt_tiles, P, KT, P).transpose(0, 3, 2, 1)
        t16 = np.ascontiguousarray(t[:, :, :j16, :].astype(np.float16))
        t8 = t[:, :, j16:j16 + 2 * dj8, :].reshape(mt_tiles, P, dj8, 2, P)
        return t16, np.ascontiguousarray(_q8(t8))

    protein = np.asarray(protein, dtype=np.float32)
    drug = np.asarray(drug, dtype=np.float32)
    in_maps = []
    rows = mt_tiles * P
    for c in range(NCORES):
        sl = slice(c * M_SH, c * M_SH + rows)
        xp16, xp8 = tile_x(protein[sl])
        xd16, xd8 = tile_x(drug[sl])
        in_maps.append({"xp16": xp16, "xd16": xd16, "xp8": xp8, "xd8": xd8,
                        "wst": wst, "wp8": wp8, "wd8": wd8})
    return in_maps


_MODULE_CACHE = {}

STRATEGY = "strassen"  # "flat" or "strassen"


def _run(protein, drug, Wv_p, Wv_d, trace=False, mt_tiles=MT_FULL):
    from concourse.bass_utils import run_bass_kernel_spmd

    key = (STRATEGY, mt_tiles)
    nc = _MODULE_CACHE.get(key)
    if nc is None:
        build = _build_strassen_module if STRATEGY == "strassen" else _build_module
        nc = _MODULE_CACHE[key] = build(mt_tiles)
    prep = _prep_strassen if STRATEGY == "strassen" else _prep_inputs
    in_maps = prep(protein, drug, Wv_p, Wv_d, mt_tiles)
    res = run_bass_kernel_spmd(nc, in_maps, list(range(NCORES)), trace=trace)
    E = np.concatenate(
        [np.asarray(r["out"], dtype=np.float32) for r in res.results], axis=0
    )
    return E, res


def kernel(
    protein,
    drug,
    mask_prot=None,
    mask_drug=None,
    Wq_p=None,
    Wk_p=None,
    Wv_p=None,
    Wq_d=None,
    Wk_d=None,
    Wv_d=None,
):
    E, _ = _run(protein, drug, Wv_p, Wv_d, trace=False)
    return np.concatenate([E, E], axis=1)


def kernel_profiled(**inputs):
    E, res = _run(
        inputs["protein"], inputs["drug"], inputs["Wv_p"], inputs["Wv_d"], trace=False
    )
    out = np.concatenate([E, E], axis=1)
    return out, res


# revision 18
# speedup vs baseline: 1.0605x; 1.0395x over previous
"""Trainium2 Bass kernel for nn_CAN_Layer_74775380623980.

Math: with sequence length L=1, softmax over the single key is exactly 1.0
and the reference's masks are overwritten with ones, so the whole cross
attention collapses to

    E   = (protein @ Wv_p + drug @ Wv_d) / 2          # [N, 2048]
    out = concat([E, E], axis=1)                      # [N, 4096]

Sharding: pure data parallel, batch N=16384 split 8 ways (2048 rows/core);
the two V projection weights are replicated.

Precision/speed split: per tensor, K-strips 0..11 (1536 of 2048) run as
fp16 matmuls; strips 12..15 run as fp8-e4m3 DoubleRow matmuls (2 K-strips
per instruction at 2x PE rate). Both paths accumulate into the same PSUM
bank at a common scale of 2^16 (x scaled by 16, weights by 0.5*4096 — all
powers of two, exact), undone by a scaled PSUM->SBUF copy. Measured
end-to-end rel_fro error vs the fp32 reference: 1.9e-2 (< 2e-2 gate).
"""

import numpy as np
import ml_dtypes

P = 128          # partitions / systolic tile
N_FULL = 16384
D = 2048         # contraction dim per tensor
HID = 2048       # output dim per projection
NCORES = 8
M_SH = N_FULL // NCORES   # 2048 rows per core
KT = D // P               # 16 k-strips per tensor
J16 = 12                  # fp16 k-strips per tensor
DJ8 = (KT - J16) // 2     # fp8 DoubleRow steps per tensor (2 strips each)
NBLK = 512                # matmul free dim (one PSUM bank of fp32)
NB = HID // NBLK          # 4 n-blocks
MT_FULL = M_SH // P       # 16 m-tiles
XSCALE = 16.0             # x pre-scale (power of two)
WSCALE = 2048.0           # 0.5 (reference's /2) * 4096 weight pre-scale
OSCALE = 1.0 / (XSCALE * 4096.0)  # PSUM de-scale = 2^-16


def _build_module(mt_tiles=MT_FULL, reps=1, xbufs=2, obufs=2, paired=True,
                  j16=J16, dj8=DJ8, swi=False, adj=False):
    """reps>1 wraps the whole body in a device-side For_i — used only for
    wall-clock benchmarking (amplifies device time above RPC noise).
    j16/dj8 override the fp16/fp8 K-split (probing). swi uses
    DoubleRowSwInterleave (host pre-interleaves the stationary layout)."""
    import concourse.bass as bass  # noqa: F401
    import concourse.mybir as mybir
    import concourse.tile as tile
    from concourse import bacc

    fp16 = mybir.dt.float16
    fp8 = mybir.dt.float8e4
    f32 = mybir.dt.float32
    DR = (mybir.MatmulPerfMode.DoubleRowSwInterleave if swi
          else mybir.MatmulPerfMode.DoubleRow)

    nc = bacc.Bacc("TRN2", target_bir_lowering=False, debug=False)

    xp16_h = nc.dram_tensor("xp16", [mt_tiles, P, max(j16, 1), P], fp16, kind="ExternalInput")
    xd16_h = nc.dram_tensor("xd16", [mt_tiles, P, max(j16, 1), P], fp16, kind="ExternalInput")
    x8_hshape = ([mt_tiles, P, max(dj8, 1), 2 * P] if swi
                 else [mt_tiles, P, max(dj8, 1), 2, P])
    xp8_h = nc.dram_tensor("xp8", x8_hshape, fp8, kind="ExternalInput")
    xd8_h = nc.dram_tensor("xd8", x8_hshape, fp8, kind="ExternalInput")
    wp16_h = nc.dram_tensor("wp16", [max(j16, 1), P, HID], fp16, kind="ExternalInput")
    wd16_h = nc.dram_tensor("wd16", [max(j16, 1), P, HID], fp16, kind="ExternalInput")
    w8_hshape = ([max(dj8, 1), P, HID, 2] if adj
                 else [max(dj8, 1), P, 2, HID])
    wp8_h = nc.dram_tensor("wp8", w8_hshape, fp8, kind="ExternalInput")
    wd8_h = nc.dram_tensor("wd8", w8_hshape, fp8, kind="ExternalInput")
    out_h = nc.dram_tensor("out", [mt_tiles * P, HID], f32, kind="ExternalOutput")
    J16_, DJ8_ = j16, dj8

    with tile.TileContext(nc) as tc:
        with (
            tc.tile_pool(name="wpool", bufs=1) as wpool,
            tc.tile_pool(name="xpool", bufs=(2 * xbufs if paired else xbufs)) as xpool,
            tc.tile_pool(name="x8pool", bufs=4) as x8pool,
            tc.tile_pool(name="stpool", bufs=1) as stpool,
            tc.tile_pool(name="opool", bufs=obufs) as opool,
            tc.tile_pool(name="psum", bufs=(1 if paired else 2), space="PSUM") as pp,
        ):
            x_tiles = {}

            x8_shape = [P, DJ8_, 2 * P] if swi else [P, DJ8_, 2, P]

            def load_x(mt):
                tp16 = xpool.tile([P, J16_, P], fp16, tag="xp16", name=f"xp16_{mt}")
                nc.sync.dma_start(tp16[:], xp16_h[mt])
                td16 = xpool.tile([P, J16_, P], fp16, tag="xd16", name=f"xd16_{mt}")
                nc.sync.dma_start(td16[:], xd16_h[mt])
                x_tiles[mt] = (tp16, td16)

            def load_x8(mt):
                tp8 = x8pool.tile(x8_shape, fp8, tag="xp8", name=f"xp8_{mt}")
                nc.sync.dma_start(tp8[:], xp8_h[mt])
                td8 = x8pool.tile(x8_shape, fp8, tag="xd8", name=f"xd8_{mt}")
                nc.sync.dma_start(td8[:], xd8_h[mt])
                x8_tiles[mt] = (tp8, td8)

            next8 = [0]

            def ensure_loads8(upto):
                while next8[0] <= min(upto, mt_tiles - 1):
                    load_x8(next8[0])
                    next8[0] += 1

            w16_sb = {}
            w8_sb = {}

            def load_w16():
                # DMA order matches PE consumption order (P0, D0, P1, D1, ...)
                w16_sb.clear()
                for j in range(J16_):
                    for t, h in (("p", wp16_h), ("d", wd16_h)):
                        tw = wpool.tile([P, HID], fp16, tag=f"w16{t}{j}",
                                        name=f"w16{t}_{j}")
                        nc.sync.dma_start(tw[:], h[j])
                        w16_sb[t, j] = tw

            def load_w8():
                w8_sb.clear()
                w8_tshape = [P, HID, 2] if adj else [P, 2, HID]
                for t, h in (("p", wp8_h), ("d", wd8_h)) if DJ8_ else ():
                    for dj in range(DJ8_):
                        tw = wpool.tile(w8_tshape, fp8, tag=f"w8{t}{dj}",
                                        name=f"w8{t}_{dj}")
                        nc.sync.dma_start(tw[:], h[dj])
                        w8_sb[t, dj] = tw

            next_load = [1]

            def ensure_loads(upto):
                while next_load[0] <= min(upto, mt_tiles - 1):
                    load_x(next_load[0])
                    next_load[0] += 1

            def fp8_pass(mt_lo, mt_hi):
                # all DoubleRow matmuls for every m-tile, partial sums staged
                # to SBUF as de-scaled fp16 (values ~ E/4, well within fp16)
                for mt0 in range(mt_lo, mt_hi, 2):
                    pair = [mt0, mt0 + 1]
                    ensure_loads8(mt0 + 3)
                    psums = {
                        (h, nb): pp.tile(
                            [P, NBLK], f32, tag=f"ps{h}_{nb}", name=f"ps8_{mt0}_{h}_{nb}"
                        )
                        for h in range(2)
                        for nb in range(NB)
                    }
                    x8t = {mt: x8_tiles.pop(mt) for mt in pair}
                    for dj in range(DJ8_):
                        for ti, t in enumerate(("p", "d")):
                            first = dj == 0 and ti == 0
                            last = dj == DJ8_ - 1 and ti == 1
                            for h, mt in enumerate(pair):
                                for nb in range(NB):
                                    nc.tensor.matmul(
                                        psums[h, nb][:],
                                        x8t[mt][ti][:, dj],
                                        w8_sb[t, dj][:, :, nb * NBLK:(nb + 1) * NBLK],
                                        start=first,
                                        stop=last,
                                        perf_mode=DR,
                                    )
                    for h, mt in enumerate(pair):
                        for nb in range(NB):
                            st = stpool.tile([P, NBLK], fp16, tag=f"st{mt % 8}_{nb}",
                                             name=f"st_{mt}_{nb}")
                            nc.vector.tensor_scalar_mul(st[:], psums[h, nb][:], OSCALE)
                            st_tiles[mt, nb] = st

            def fp16_pass(mt_lo, mt_hi):
                for mt0 in range(mt_lo, mt_hi, 2):
                    pair = [mt0, mt0 + 1]
                    if J16_:
                        ensure_loads(mt0 + 1 + 2 * (xbufs - 1))
                        xt = {mt: x_tiles.pop(mt) for mt in pair}
                    psums = {
                        (h, nb): pp.tile(
                            [P, NBLK], f32, tag=f"ps{h}_{nb}", name=f"ps_{mt0}_{h}_{nb}"
                        )
                        for h in range(2)
                        for nb in range(NB)
                    }
                    for j in range(J16_):
                        for ti, t in enumerate(("p", "d")):
                            for h, mt in enumerate(pair):
                                for nb in range(NB):
                                    nc.tensor.matmul(
                                        psums[h, nb][:],
                                        xt[mt][ti][:, j, :],
                                        w16_sb[t, j][:, nb * NBLK:(nb + 1) * NBLK],
                                        start=(j == 0 and ti == 0),
                                        stop=(j == J16_ - 1 and ti == 1),
                                    )
                    for h, mt in enumerate(pair):
                        out_t = opool.tile([P, HID], f32, tag="out", name=f"out_{mt}")
                        for nb in range(NB):
                            if J16_ == 0:
                                nc.vector.tensor_copy(
                                    out_t[:, nb * NBLK:(nb + 1) * NBLK],
                                    st_tiles[mt, nb][:],
                                )
                            elif DJ8_:
                                nc.vector.scalar_tensor_tensor(
                                    out_t[:, nb * NBLK:(nb + 1) * NBLK],
                                    psums[h, nb][:],
                                    OSCALE,
                                    st_tiles[mt, nb][:],
                                    op0=mybir.AluOpType.mult,
                                    op1=mybir.AluOpType.add,
                                )
                            else:
                                nc.vector.tensor_scalar_mul(
                                    out_t[:, nb * NBLK:(nb + 1) * NBLK],
                                    psums[h, nb][:],
                                    OSCALE,
                                )
                        nc.sync.dma_start(out_h[mt * P:(mt + 1) * P, :], out_t[:])

            def body():
                assert mt_tiles % 4 == 0
                st_tiles.clear()
                x_tiles.clear()
                x8_tiles.clear()
                next8[0] = 0
                next_load[0] = 1
                if DJ8_:
                    load_w8()
                    ensure_loads8(3)
                if J16_:
                    load_x(0)
                    load_w16()
                half = mt_tiles // 2
                if DJ8_:
                    fp8_pass(0, half)
                    fp16_pass(0, half)
                    fp8_pass(half, mt_tiles)
                    fp16_pass(half, mt_tiles)
                else:
                    fp16_pass(0, mt_tiles)

            st_tiles = {}
            x8_tiles = {}
            if reps == 1:
                body()
            else:
                with tc.For_i(0, reps, 1):
                    body()

    nc.compile()
    return nc


def _q8(a):
    return a.astype(ml_dtypes.float8_e4m3)


def _prep_inputs(protein, drug, Wv_p, Wv_d, mt_tiles=MT_FULL,
                 j16=J16, dj8=DJ8, swi=False, adj=False):
    """Host-side shard + transpose-tile + dtype split/cast."""
    kcut = j16 * P

    def prep_w(W):
        W = np.asarray(W, dtype=np.float32) * WSCALE
        if j16:
            w16 = np.ascontiguousarray(
                W[:kcut].reshape(j16, P, HID).astype(np.float16))
        else:
            w16 = np.zeros((1, P, HID), np.float16)
        if dj8:
            # w8[dj, p, i, n] = W[kcut + (2dj+i)*P + p, n]
            w8 = W[kcut:kcut + dj8 * 2 * P].reshape(dj8, 2, P, HID)
            w8 = w8.transpose(0, 2, 1, 3)          # [dj, p, i, n]
            if adj:
                w8 = w8.transpose(0, 1, 3, 2)      # [dj, p, n, i]
            w8 = np.ascontiguousarray(_q8(w8))
        else:
            shape = (1, P, HID, 2) if adj else (1, P, 2, HID)
            w8 = np.zeros(shape, ml_dtypes.float8_e4m3)
        return w16, w8

    wp16, wp8 = prep_w(Wv_p)
    wd16, wd8 = prep_w(Wv_d)

    def tile_x(x):
        x = x * XSCALE
        # [rows, D] -> [mt, p, j, m]: t[mt,p,j,m] = x[mt*P+m, j*P+p]
        t = x.reshape(mt_tiles, P, KT, P).transpose(0, 3, 2, 1)
        if j16:
            t16 = np.ascontiguousarray(t[:, :, :j16, :].astype(np.float16))
        else:
            t16 = np.zeros((mt_tiles, P, 1, P), np.float16)
        if dj8:
            t8 = t[:, :, j16:j16 + 2 * dj8, :].reshape(mt_tiles, P, dj8, 2, P)
            if swi:
                # stored[p, dj, 2m'+i] = plane_i[p, 127-m'] (interleaved,
                # columns reversed) per DoubleRowSwInterleave convention
                rev = t8[:, :, :, :, ::-1]                   # [mt,p,dj,i,m']
                t8 = rev.transpose(0, 1, 2, 4, 3).reshape(mt_tiles, P, dj8, 2 * P)
            t8 = np.ascontiguousarray(_q8(t8))
        else:
            shape = (mt_tiles, P, 1, 2 * P) if swi else (mt_tiles, P, 1, 2, P)
            t8 = np.zeros(shape, ml_dtypes.float8_e4m3)
        return t16, t8

    protein = np.asarray(protein, dtype=np.float32)
    drug = np.asarray(drug, dtype=np.float32)
    in_maps = []
    rows = mt_tiles * P
    for c in range(NCORES):
        sl = slice(c * M_SH, c * M_SH + rows)
        xp16, xp8 = tile_x(protein[sl])
        xd16, xd8 = tile_x(drug[sl])
        in_maps.append(
            {
                "xp16": xp16, "xd16": xd16, "xp8": xp8, "xd8": xd8,
                "wp16": wp16, "wd16": wd16, "wp8": wp8, "wd8": wd8,
            }
        )
    return in_maps




# ---------------------------------------------------------------------------
# Strassen-1 on the fp16 portion (level-1 split of the [2048 x 3072] fp16
# GEMM into 7 products of [1024 x 1536] @ [1536 x 1024]), fp8 DoubleRow
# strips handled in separate per-column-group passes with fp16 staging.
# B-side Strassen combos are precomputed on the host and shipped as `wst`.

def _build_strassen_module(mt_tiles=MT_FULL, reps=1, j16=J16, dj8=DJ8):
    import concourse.bass as bass  # noqa: F401
    import concourse.mybir as mybir
    import concourse.tile as tile
    from concourse import bacc

    fp16 = mybir.dt.float16
    fp8 = mybir.dt.float8e4
    f32 = mybir.dt.float32
    DR = mybir.MatmulPerfMode.DoubleRow
    ADD = mybir.AluOpType.add
    SUB = mybir.AluOpType.subtract
    MULT = mybir.AluOpType.mult

    assert mt_tiles == 16 and j16 >= 1 and dj8 >= 0
    MH = mt_tiles // 2            # 8 row-tiles per row-half

    nc = bacc.Bacc("TRN2", target_bir_lowering=False, debug=False)

    xp16_h = nc.dram_tensor("xp16", [mt_tiles, P, j16, P], fp16, kind="ExternalInput")
    xd16_h = nc.dram_tensor("xd16", [mt_tiles, P, j16, P], fp16, kind="ExternalInput")
    d8 = max(dj8, 1)
    xp8_h = nc.dram_tensor("xp8", [mt_tiles, P, d8, 2, P], fp8, kind="ExternalInput")
    xd8_h = nc.dram_tensor("xd8", [mt_tiles, P, d8, 2, P], fp8, kind="ExternalInput")
    wst_h = nc.dram_tensor("wst", [7, j16, P, HID // 2], fp16, kind="ExternalInput")
    wp8_h = nc.dram_tensor("wp8", [d8, P, 2, HID], fp8, kind="ExternalInput")
    wd8_h = nc.dram_tensor("wd8", [d8, P, 2, HID], fp8, kind="ExternalInput")
    out_h = nc.dram_tensor("out", [mt_tiles * P, HID], f32, kind="ExternalOutput")

    with tile.TileContext(nc) as tc:
        with (
            tc.tile_pool(name="bst", bufs=1) as bstpool,
            tc.tile_pool(name="w8pool", bufs=1) as w8pool,
            tc.tile_pool(name="xpool", bufs=2) as xpool,
            tc.tile_pool(name="x8pool", bufs=8) as x8pool,
            tc.tile_pool(name="apool", bufs=1) as apool,
            tc.tile_pool(name="stpool", bufs=1) as stpool,
            tc.tile_pool(name="opool", bufs=2) as opool,
            tc.tile_pool(name="psum", bufs=1, space="PSUM") as pp,
        ):
            w8_sb = {}

            def load_w8():
                for t, h in (("p", wp8_h), ("d", wd8_h)):
                    for dj in range(dj8):
                        tw = w8pool.tile([P, 2, HID], fp8, tag=f"w8{t}{dj}",
                                         name=f"w8{t}_{dj}")
                        nc.sync.dma_start(tw[:], h[dj])
                        w8_sb[t, dj] = tw

            def load_bst(u):
                # 7 B-combo slices [128, j16, 512] for this 512-col group
                tiles = []
                for i in range(7):
                    tw = bstpool.tile([P, j16, NBLK], fp16, tag=f"bst{i}",
                                      name=f"bst{i}_{u}")
                    for j in range(j16):
                        nc.sync.dma_start(
                            tw[:, j, :], wst_h[i, j][:, u * NBLK:(u + 1) * NBLK])
                    tiles.append(tw)
                return tiles

            x_tiles = {}
            next_x = [0]

            def load_x16(rp, gen):
                tags = (("pt", xp16_h, rp), ("pb", xp16_h, MH + rp),
                        ("dt", xd16_h, rp), ("db", xd16_h, MH + rp))
                tt = {}
                for tag, h, mt in tags:
                    t = xpool.tile([P, j16, P], fp16, tag=tag, name=f"{tag}_{gen}_{rp}")
                    nc.sync.dma_start(t[:], h[mt])
                    tt[tag] = t
                x_tiles[rp] = tt

            def ensure_x16(upto, gen):
                while next_x[0] <= min(upto, MH - 1):
                    load_x16(next_x[0], gen)
                    next_x[0] += 1

            x8_tiles = {}
            next_x8 = [0]

            def load_x8(mt, gen):
                tp8 = x8pool.tile([P, dj8, 2, P], fp8, tag="xp8",
                                  name=f"xp8_{gen}_{mt}")
                nc.sync.dma_start(tp8[:], xp8_h[mt])
                td8 = x8pool.tile([P, dj8, 2, P], fp8, tag="xd8",
                                  name=f"xd8_{gen}_{mt}")
                nc.sync.dma_start(td8[:], xd8_h[mt])
                x8_tiles[mt] = (tp8, td8)

            def ensure_x8(upto, gen):
                while next_x8[0] <= min(upto, mt_tiles - 1):
                    load_x8(next_x8[0], gen)
                    next_x8[0] += 1

            st_tiles = {}

            def fp8_pass(u, gen):
                # partials for column groups u (left half) and 2+u (right),
                # all 16 row tiles; staged to SBUF as de-scaled fp16
                next_x8[0] = 0
                x8_tiles.clear()
                for q0 in range(0, mt_tiles, 4):
                    quad = list(range(q0, q0 + 4))
                    ensure_x8(q0 + 7, gen)
                    x8t = {mt: x8_tiles.pop(mt) for mt in quad}
                    psums = {}
                    for qi, mt in enumerate(quad):
                        for si, nbg in enumerate((u, 2 + u)):
                            psums[mt, nbg] = pp.tile(
                                [P, NBLK], f32, tag=f"ps{2 * qi + si + 1}",
                                name=f"ps8_{gen}_{mt}_{nbg}")
                    for dj in range(dj8):
                        for ti, t in enumerate(("p", "d")):
                            first = dj == 0 and ti == 0
                            last = dj == dj8 - 1 and ti == 1
                            for mt in quad:
                                for nbg in (u, 2 + u):
                                    nc.tensor.matmul(
                                        psums[mt, nbg][:],
                                        x8t[mt][ti][:, dj],
                                        w8_sb[t, dj][:, :, nbg * NBLK:(nbg + 1) * NBLK],
                                        start=first,
                                        stop=last,
                                        perf_mode=DR,
                                    )
                    for mt in quad:
                        for si, nbg in enumerate((u, 2 + u)):
                            st = stpool.tile([P, NBLK], fp16, tag=f"st{mt}_{si}",
                                             name=f"st_{gen}_{mt}_{nbg}")
                            nc.vector.tensor_scalar_mul(
                                st[:], psums[mt, nbg][:], OSCALE)
                            st_tiles[mt, si] = st

            def strassen_pass(u, bst, gen):
                next_x[0] = 0
                x_tiles.clear()
                for rp in range(MH):
                    ensure_x16(rp + 1, gen)
                    xt = x_tiles.pop(rp)
                    pt, pb, dt, db = xt["pt"], xt["pb"], xt["dt"], xt["db"]
                    combos = [("a1", pt, db, ADD), ("a2", pb, db, ADD),
                              ("a5", pt, dt, ADD), ("a6", pb, pt, SUB),
                              ("a7", dt, db, SUB)]
                    at = {}
                    for tag, i0, i1, op in combos:
                        t = apool.tile([P, j16, P], fp16, tag=tag,
                                       name=f"{tag}_{gen}_{rp}")
                        if op is SUB:
                            # a6 = pb - pt, a7 = dt - db
                            nc.vector.tensor_tensor(t[:], i0[:], i1[:], op)
                        else:
                            nc.vector.tensor_tensor(t[:], i0[:], i1[:], op)
                        at[tag] = t
                    stats = [at["a1"], at["a2"], pt, db, at["a5"], at["a6"], at["a7"]]
                    ps = []
                    for i in range(7):
                        p_t = pp.tile([P, NBLK], f32, tag=f"ps{i + 1}",
                                      name=f"psS_{gen}_{rp}_{i}")
                        for j in range(j16):
                            nc.tensor.matmul(
                                p_t[:], stats[i][:, j, :], bst[i][:, j, :],
                                start=(j == 0), stop=(j == j16 - 1))
                        ps.append(p_t)
                    m1, m2, m3, m4, m5, m6, m7 = ps
                    o = {}
                    for key, mt_row, col in (("o11", rp, u), ("o12", rp, 2 + u),
                                             ("o21", MH + rp, u),
                                             ("o22", MH + rp, 2 + u)):
                        o[key] = opool.tile([P, NBLK], f32, tag=key,
                                            name=f"{key}_{gen}_{rp}")
                    # DVE may read at most one PSUM operand per instruction:
                    # copy psum->sbuf once, then accumulate psum terms one at
                    # a time. Ordered to free psum banks early for the next
                    # iteration's products.
                    nc.vector.tensor_copy(o["o11"][:], m1[:])
                    nc.vector.tensor_scalar_mul(o["o22"][:], m2[:], -1.0)
                    nc.vector.tensor_tensor(o["o22"][:], o["o22"][:], m1[:], ADD)
                    nc.vector.tensor_copy(o["o21"][:], m2[:])
                    nc.vector.tensor_tensor(o["o11"][:], o["o11"][:], m4[:], ADD)
                    nc.vector.tensor_tensor(o["o21"][:], o["o21"][:], m4[:], ADD)
                    nc.vector.tensor_copy(o["o12"][:], m3[:])
                    nc.vector.tensor_tensor(o["o22"][:], o["o22"][:], m3[:], ADD)
                    nc.vector.tensor_tensor(o["o12"][:], o["o12"][:], m5[:], ADD)
                    nc.vector.tensor_tensor(o["o11"][:], o["o11"][:], m5[:], SUB)
                    nc.vector.tensor_tensor(o["o22"][:], o["o22"][:], m6[:], ADD)
                    nc.vector.tensor_tensor(o["o11"][:], o["o11"][:], m7[:], ADD)
                    for key, mt_row, si in (("o11", rp, 0), ("o12", rp, 1),
                                            ("o21", MH + rp, 0),
                                            ("o22", MH + rp, 1)):
                        nbg = si * 2 + u
                        if dj8:
                            nc.vector.scalar_tensor_tensor(
                                o[key][:], o[key][:], OSCALE, st_tiles[mt_row, si][:],
                                op0=MULT, op1=ADD)
                        else:
                            nc.vector.tensor_scalar_mul(
                                o[key][:], o[key][:], OSCALE)
                        nc.sync.dma_start(
                            out_h[mt_row * P:(mt_row + 1) * P,
                                  nbg * NBLK:(nbg + 1) * NBLK],
                            o[key][:])

            def body(rep):
                st_tiles.clear()
                if dj8:
                    load_w8()
                for u in (0, 1):
                    gen = f"{rep}_{u}"
                    if dj8:
                        fp8_pass(u, gen)
                    bst = load_bst(u)
                    strassen_pass(u, bst, gen)

            if reps == 1:
                body(0)
            else:
                with tc.For_i(0, reps, 1):
                    body("r")

    nc.compile()
    return nc


def _prep_strassen(protein, drug, Wv_p, Wv_d, mt_tiles=MT_FULL, j16=J16, dj8=DJ8):
    kcut = j16 * P
    HH = HID // 2

    def prep_w8(W):
        if not dj8:
            return np.zeros((1, P, 2, HID), ml_dtypes.float8_e4m3)
        W = np.asarray(W, dtype=np.float32) * WSCALE
        w8 = W[kcut:kcut + dj8 * 2 * P].reshape(dj8, 2, P, HID).transpose(0, 2, 1, 3)
        return np.ascontiguousarray(_q8(w8))

    WSp = np.asarray(Wv_p, dtype=np.float32) * WSCALE
    WSd = np.asarray(Wv_d, dtype=np.float32) * WSCALE
    B11, B12 = WSp[:kcut, :HH], WSp[:kcut, HH:]
    B21, B22 = WSd[:kcut, :HH], WSd[:kcut, HH:]
    combos = [B11 + B22, B11, B12 - B22, B21 - B11, B22, B11 + B12, B21 + B22]
    wst = np.stack([c.reshape(j16, P, HH) for c in combos]).astype(np.float16)
    wst = np.ascontiguousarray(wst)
    wp8 = prep_w8(Wv_p)
    wd8 = prep_w8(Wv_d)

    def tile_x(x):
        x = x * XSCALE
        t = x.reshape(mt_tiles, P, KT, P).transpose(0, 3, 2, 1)
        t16 = np.ascontiguousarray(t[:, :, :j16, :].astype(np.float16))
        if not dj8:
            return t16, np.zeros((mt_tiles, P, 1, 2, P), ml_dtypes.float8_e4m3)
        t8 = t[:, :, j16:j16 + 2 * dj8, :].reshape(mt_tiles, P, dj8, 2, P)
        return t16, np.ascontiguousarray(_q8(t8))

    protein = np.asarray(protein, dtype=np.float32)
    drug = np.asarray(drug, dtype=np.float32)
    in_maps = []
    rows = mt_tiles * P
    for c in range(NCORES):
        sl = slice(c * M_SH, c * M_SH + rows)
        xp16, xp8 = tile_x(protein[sl])
        xd16, xd8 = tile_x(drug[sl])
        in_maps.append({"xp16": xp16, "xd16": xd16, "xp8": xp8, "xd8": xd8,
                        "wst": wst, "wp8": wp8, "wd8": wd8})
    return in_maps


_MODULE_CACHE = {}

STRATEGY = "strassen"      # "flat" or "strassen"
STRASSEN_CFG = (12, 2)     # (fp16 strips, fp8 DoubleRow steps) per tensor


def _run(protein, drug, Wv_p, Wv_d, trace=False, mt_tiles=MT_FULL):
    from concourse.bass_utils import run_bass_kernel_spmd

    key = (STRATEGY, STRASSEN_CFG, mt_tiles)
    nc = _MODULE_CACHE.get(key)
    if nc is None:
        if STRATEGY == "strassen":
            nc = _build_strassen_module(
                mt_tiles, j16=STRASSEN_CFG[0], dj8=STRASSEN_CFG[1])
        else:
            nc = _build_module(mt_tiles)
        _MODULE_CACHE[key] = nc
    if STRATEGY == "strassen":
        in_maps = _prep_strassen(protein, drug, Wv_p, Wv_d, mt_tiles,
                                 j16=STRASSEN_CFG[0], dj8=STRASSEN_CFG[1])
    else:
        in_maps = _prep_inputs(protein, drug, Wv_p, Wv_d, mt_tiles)
    res = run_bass_kernel_spmd(nc, in_maps, list(range(NCORES)), trace=trace)
    E = np.concatenate(
        [np.asarray(r["out"], dtype=np.float32) for r in res.results], axis=0
    )
    return E, res


def kernel(
    protein,
    drug,
    mask_prot=None,
    mask_drug=None,
    Wq_p=None,
    Wk_p=None,
    Wv_p=None,
    Wq_d=None,
    Wk_d=None,
    Wv_d=None,
):
    E, _ = _run(protein, drug, Wv_p, Wv_d, trace=False)
    return np.concatenate([E, E], axis=1)


def kernel_profiled(**inputs):
    E, res = _run(
        inputs["protein"], inputs["drug"], inputs["Wv_p"], inputs["Wv_d"], trace=False
    )
    out = np.concatenate([E, E], axis=1)
    return out, res
